# revision 25
# baseline (speedup 1.0000x reference)
"""Multi-head attention (B=2, S=2048, D=1024, H=16, Dk=64) on 8 NeuronCores.

Sharding: 2-way data parallel over batch x 4-way tensor parallel over heads.
Core c = 4*b + g handles batch b, head group g (4 heads = 256 cols).
W_o is row-sliced; the 4 partial outputs per batch are summed on host (+bo).

Per-core layout (all transposed — zero on-device transposes):
  host passes xT = x[b].T as float32r (TF32-like, rounded host-side)
  QT/KT = Wg^T @ xT computed as [256, 2048] (dk on partitions)
  scores^T = K @ Q^T per head; softmax along partitions (exp without max —
  scores are O(1) by construction); rowsums via 64 ones-columns appended to
  V in the P@V matmul (M=128: rows 0-63 = O^T unnorm, 64-127 = rowsum x64)
  out_partial = O^T.T @ Wo_rows directly from the O^T layout.
"""
import numpy as np
from contextlib import ExitStack

import concourse.bass as bass
import concourse.mybir as mybir
import concourse.tile as tile
from concourse import bacc
from concourse.bass_utils import run_bass_kernel_spmd

F32 = mybir.dt.float32
F32R = mybir.dt.float32r

B, S, D = 2, 2048, 1024
H, DK = 16, 64
P = 128
KO = D // P          # 8 contraction subtiles for the projections
W = 256              # local width (4 heads x 64)
MT = W // P          # 2 m-tiles = head pairs
NQ = S // 512        # 4 sq chunks
NSK = S // P         # 16 sk chunks
SCALE = 1.0 / 8.0    # 1/sqrt(DK)

_CACHE = {}


def to_f32r(x: np.ndarray) -> np.ndarray:
    """Round fp32 to the fp32r bit layout (8e11m in the top 20 bits, RNE)."""
    u = np.ascontiguousarray(x, dtype=np.float32).view(np.uint32)
    r = u + np.uint32(0x7FF) + ((u >> np.uint32(12)) & np.uint32(1))
    r &= np.uint32(0xFFFFF000)
    return r.view(np.float32)


def build_nc(repeat=1):
    nc = bacc.Bacc("TRN2", target_bir_lowering=False, debug=False, num_devices=8)
    xT = nc.dram_tensor("xT", [D, S], F32R, kind="ExternalInput").ap()
    wq = nc.dram_tensor("wq", [D, W], F32R, kind="ExternalInput").ap()
    wk = nc.dram_tensor("wk", [D, W], F32R, kind="ExternalInput").ap()
    wv = nc.dram_tensor("wv", [D, W], F32R, kind="ExternalInput").ap()
    wo = nc.dram_tensor("wo", [W, D], F32R, kind="ExternalInput").ap()
    bq = nc.dram_tensor("bq", [P, MT], F32, kind="ExternalInput").ap()
    bk = nc.dram_tensor("bk", [P, MT], F32, kind="ExternalInput").ap()
    bv = nc.dram_tensor("bv", [P, W], F32, kind="ExternalInput").ap()
    out = nc.dram_tensor("out", [S, D], F32, kind="ExternalOutput").ap()

    xT_r = xT.rearrange("(ko p) s -> p ko s", p=P)
    wq_r = wq.rearrange("(ko p) w -> p ko w", p=P)
    wk_r = wk.rearrange("(ko p) w -> p ko w", p=P)
    wv_r = wv.rearrange("(ko p) w -> p ko w", p=P)
    wo_r = wo.rearrange("(ko p) d -> p ko d", p=P)

    with tile.TileContext(nc) as tc, ExitStack() as ctx:
        sb = ctx.enter_context(tc.tile_pool(name="sb", bufs=1))
        xp = ctx.enter_context(tc.tile_pool(name="xp", bufs=2))
        ptp = ctx.enter_context(tc.tile_pool(name="ptp", bufs=4))
        cp = ctx.enter_context(tc.tile_pool(name="cp", bufs=4))
        ps = ctx.enter_context(tc.tile_pool(name="ps", bufs=1, space="PSUM"))

        # ---- resident tiles (issue order matters: first matmuls need
        #      only wq + the first xT quarter; wo only at output proj) ----
        xq_tiles = [xp.tile([P, KO, 512], F32R, tag="xq", name=f"xq{i}")
                    for i in range(4)]

        def load_xq(qtr):
            sq = slice(qtr * 512, (qtr + 1) * 512)
            nc.sync.dma_start(xq_tiles[qtr][:, 0:KO // 2, :],
                              xT_r[:, 0:KO // 2, sq])
            nc.sync.dma_start(xq_tiles[qtr][:, KO // 2:KO, :],
                              xT_r[:, KO // 2:KO, sq])

        wq_t = sb.tile([P, KO, W], F32R)
        nc.sync.dma_start(wq_t[:], wq_r)
        load_xq(0)
        bq_t = sb.tile([P, MT], F32)
        nc.sync.dma_start(bq_t[:], bq)
        wk_t = sb.tile([P, KO, W], F32R)
        nc.sync.dma_start(wk_t[:], wk_r)
        bk_t = sb.tile([P, MT], F32)
        nc.sync.dma_start(bk_t[:], bk)
        wv_t = sb.tile([P, KO, W], F32R)
        nc.sync.dma_start(wv_t[:], wv_r)
        bv_t = sb.tile([P, W], F32)
        nc.sync.dma_start(bv_t[:], bv)
        wo_t = sb.tile([P, MT, D], F32R)
        nc.sync.dma_start(wo_t[:], wo_r)

        qt_t = sb.tile([P, MT, S], F32R)      # [dk-pair, hp, seq]
        kt_t = sb.tile([P, MT, S], F32R)
        # vaug: per sk chunk, per head h: cols h*128+0:64 = V_h + bv,
        #       cols h*128+64:128 = ones (rowsum trick)
        vaug_t = sb.tile([P, NSK, 4 * P], F32R)
        ones_view = vaug_t[:].rearrange("p s (h c) -> p s h c", c=P)[:, :, :, DK:P]
        nc.vector.memset(ones_view.bitcast(F32), 1.0)
        ot_t = sb.tile([P, MT, S], F32R)      # normalized O^T

        # One (q, hp) attention block as a resumable generator of s-steps so
        # the first block can interleave with the projection quarters.
        _uid = [0]

        def attn_block_steps(q, hp):
            _uid[0] += 1
            u = _uid[0]
            sq = slice(q * 512, (q + 1) * 512)
            oA = ps.tile([P, 512], F32, tag="oA", bufs=1, name=f"oA{u}_{q}_{hp}")
            oB = ps.tile([P, 512], F32, tag="oB", bufs=1, name=f"oB{u}_{q}_{hp}")
            hA, hB = 2 * hp, 2 * hp + 1
            pts = {}

            def attn_v(s):
                pt = pts.pop(s)
                nc.tensor.matmul(oA[:], vaug_t[:, s, hA * P:(hA + 1) * P],
                                 pt[:, 0:512],
                                 start=(s == 0), stop=(s == NSK - 1))
                nc.tensor.matmul(oB[:], vaug_t[:, s, hB * P:(hB + 1) * P],
                                 pt[:, 512:1024],
                                 start=(s == 0), stop=(s == NSK - 1))

            for s in range(NSK):
                sp = ps.tile([P, 1024], F32, tag="spair", bufs=2,
                             name=f"sp{u}_{q}_{hp}_{s}")
                ks = slice(s * P, (s + 1) * P)
                nc.tensor.matmul(sp[:, 0:512], kt_t[0:64, hp, ks],
                                 qt_t[0:64, hp, sq], start=True, stop=True)
                nc.tensor.matmul(sp[:, 512:1024], kt_t[64:P, hp, ks],
                                 qt_t[64:P, hp, sq], start=True, stop=True)
                pt = ptp.tile([P, 1024], F32R, tag="pt", name=f"pt{u}_{q}_{hp}_{s}")
                nc.scalar.activation(pt[:], sp[:],
                                     mybir.ActivationFunctionType.Exp,
                                     bias=0.0, scale=SCALE)
                pts[s] = pt
                if s >= 1:
                    attn_v(s - 1)
                yield s
            attn_v(NSK - 1)
            rsA = cp.tile([64, 512], F32, tag="rs", bufs=2, name=f"rA{u}_{q}_{hp}")
            nc.vector.reciprocal(rsA[:], oA[64:P, :])
            nc.vector.tensor_mul(ot_t[0:64, hp, sq], oA[0:64, :], rsA[:])
            rsB = cp.tile([64, 512], F32, tag="rs", bufs=2, name=f"rB{u}_{q}_{hp}")
            nc.vector.reciprocal(rsB[:], oB[64:P, :])
            nc.vector.tensor_mul(ot_t[64:P, hp, sq], oB[0:64, :], rsB[:])
            yield NSK

        # ---- phase 1: projections, quarter of seq at a time ----
        for _rep in range(repeat):
          first_blk = attn_block_steps(0, 0)
          for qtr in range(4):
              sq = slice(qtr * 512, (qtr + 1) * 512)
              if qtr > 0 or _rep > 0:
                  if _rep > 0:
                      xq_tiles[qtr] = xp.tile([P, KO, 512], F32R, tag="xq",
                                              name=f"xqr{qtr}")
                  load_xq(qtr)
              xq_t = xq_tiles[qtr]
              for dst, wt, bt in ((qt_t, wq_t, bq_t), (kt_t, wk_t, bk_t)):
                  for m in range(MT):
                      pp = ps.tile([P, 512], F32, tag="po", bufs=2)
                      for k in range(KO):
                          nc.tensor.matmul(pp[:], wt[:, k, m * P:(m + 1) * P],
                                           xq_t[:, k, :],
                                           start=(k == 0), stop=(k == KO - 1))
                      nc.vector.tensor_scalar_add(dst[:, m, sq], pp[:], bt[:, m:m + 1])
              for st in range(4):  # V for 4 seq-tiles of 128 within the quarter
                  pv = ps.tile([P, W], F32, tag="po", bufs=2)
                  for k in range(KO):
                      nc.tensor.matmul(pv[:], xq_t[:, k, st * P:(st + 1) * P],
                                       wv_t[:, k, :],
                                       start=(k == 0), stop=(k == KO - 1))
                  skc = qtr * 4 + st
                  vdst = vaug_t[:, skc, :].rearrange("p (h c) -> p h c", c=P)[:, :, 0:DK]
                  nc.vector.tensor_add(
                      vdst,
                      pv[:].rearrange("p (h c) -> p h c", c=DK),
                      bv_t[:].rearrange("p (h c) -> p h c", c=DK))
              for _ in range(4):
                  next(first_blk, None)

          # ---- phase 2+3: attention + output projection ----
          for q in range(NQ):
              for hp in range(MT):
                  if (q, hp) == (0, 0):
                      for _ in first_blk:
                          pass
                  else:
                      for _ in attn_block_steps(q, hp):
                          pass
              # output projection for the 4 seq-tiles covered by this q chunk
              for st in range(4):
                  mo = q * 4 + st
                  for n in range(2):
                      po = ps.tile([P, 512], F32, tag="po", bufs=2)
                      for k in range(MT):
                          nc.tensor.matmul(
                              po[:], ot_t[:, k, mo * P:(mo + 1) * P],
                              wo_t[:, k, n * 512:(n + 1) * 512],
                              start=(k == 0), stop=(k == MT - 1))
                      ob = cp.tile([P, 512], F32, tag="ob")
                      nc.vector.tensor_copy(ob[:], po[:])
                      nc.sync.dma_start(
                          out[mo * P:(mo + 1) * P, n * 512:(n + 1) * 512], ob[:])
    nc.compile()
    return nc


def _prep_inputs(x, Wq, bq, Wk, bk, Wv, bv, Wo, bo):
    in_maps = []
    xTb = [to_f32r(np.ascontiguousarray(x[b].T)) for b in range(B)]
    for c in range(8):
        b, g = c // 4, c % 4
        cs = slice(g * W, (g + 1) * W)
        in_maps.append({
            "xT": xTb[b],
            "wq": to_f32r(Wq[:, cs]),
            "wk": to_f32r(Wk[:, cs]),
            "wv": to_f32r(Wv[:, cs]),
            "wo": to_f32r(Wo[cs, :]),
            "bq": np.ascontiguousarray(bq[cs].reshape(MT, P).T),
            "bk": np.ascontiguousarray(bk[cs].reshape(MT, P).T),
            "bv": np.tile(bv[cs], (P, 1)),
        })
    return in_maps


def kernel(x, Wq, bq, Wk, bk, Wv, bv, Wo, bo):
    x = np.asarray(x, dtype=np.float32)
    Wq, bq = np.asarray(Wq, np.float32), np.asarray(bq, np.float32)
    Wk, bk = np.asarray(Wk, np.float32), np.asarray(bk, np.float32)
    Wv, bv = np.asarray(Wv, np.float32), np.asarray(bv, np.float32)
    Wo, bo = np.asarray(Wo, np.float32), np.asarray(bo, np.float32)

    if "nc" not in _CACHE:
        _CACHE["nc"] = build_nc()
    nc = _CACHE["nc"]

    in_maps = _prep_inputs(x, Wq, bq, Wk, bk, Wv, bv, Wo, bo)
    res = run_bass_kernel_spmd(nc, in_maps, core_ids=list(range(8))).results

    out = np.empty((B, S, D), dtype=np.float32)
    for b in range(B):
        acc = res[4 * b]["out"].copy()
        for g in range(1, 4):
            acc += res[4 * b + g]["out"]
        out[b] = acc + bo
    return out



# revision 30
# speedup vs baseline: 1.0185x; 1.0185x over previous
"""Multi-head attention (B=2, S=2048, D=1024, H=16, Dk=64) on 8 NeuronCores.

Sharding: 2-way data parallel over batch x 4-way tensor parallel over heads.
Core c = 4*b + g handles batch b, head group g (4 heads = 256 cols).
W_o is row-sliced; the 4 partial outputs per batch are summed on host (+bo).

Per-core layout (all transposed — zero on-device transposes):
  host passes xT = x[b].T as float32r (TF32-like, rounded host-side)
  QT/KT = Wg^T @ xT computed as [256, 2048] (dk on partitions)
  scores^T = K @ Q^T per head; softmax along partitions (exp without max —
  scores are O(1) by construction); rowsums via 64 ones-columns appended to
  V in the P@V matmul (M=128: rows 0-63 = O^T unnorm, 64-127 = rowsum x64)
  out_partial = O^T.T @ Wo_rows directly from the O^T layout.
"""
import numpy as np
from contextlib import ExitStack

import concourse.bass as bass
import concourse.mybir as mybir
import concourse.tile as tile
from concourse import bacc
from concourse.bass_utils import run_bass_kernel_spmd

F32 = mybir.dt.float32
F32R = mybir.dt.float32r

B, S, D = 2, 2048, 1024
H, DK = 16, 64
P = 128
KO = D // P          # 8 contraction subtiles for the projections
W = 256              # local width (4 heads x 64)
MT = W // P          # 2 m-tiles = head pairs
NQ = S // 512        # 4 sq chunks
NSK = S // P         # 16 sk chunks
SCALE = 1.0 / 8.0    # 1/sqrt(DK)

_CACHE = {}


def to_f32r(x: np.ndarray) -> np.ndarray:
    """Round fp32 to the fp32r bit layout (8e11m in the top 20 bits, RNE)."""
    u = np.ascontiguousarray(x, dtype=np.float32).view(np.uint32)
    r = u + np.uint32(0x7FF) + ((u >> np.uint32(12)) & np.uint32(1))
    r &= np.uint32(0xFFFFF000)
    return r.view(np.float32)


def build_nc(repeat=1):
    nc = bacc.Bacc("TRN2", target_bir_lowering=False, debug=False, num_devices=8)
    xT = nc.dram_tensor("xT", [D, S], F32R, kind="ExternalInput").ap()
    wq = nc.dram_tensor("wq", [D, W], F32R, kind="ExternalInput").ap()
    wk = nc.dram_tensor("wk", [D, W], F32R, kind="ExternalInput").ap()
    wv = nc.dram_tensor("wv", [D, W], F32R, kind="ExternalInput").ap()
    wo = nc.dram_tensor("wo", [W, D], F32R, kind="ExternalInput").ap()
    bq = nc.dram_tensor("bq", [P, MT], F32, kind="ExternalInput").ap()
    bk = nc.dram_tensor("bk", [P, MT], F32, kind="ExternalInput").ap()
    bv = nc.dram_tensor("bv", [P, W], F32, kind="ExternalInput").ap()
    out = nc.dram_tensor("out", [S, D], F32, kind="ExternalOutput").ap()

    xT_r = xT.rearrange("(ko p) s -> p ko s", p=P)
    wq_r = wq.rearrange("(ko p) w -> p ko w", p=P)
    wk_r = wk.rearrange("(ko p) w -> p ko w", p=P)
    wv_r = wv.rearrange("(ko p) w -> p ko w", p=P)
    wo_r = wo.rearrange("(ko p) d -> p ko d", p=P)

    with tile.TileContext(nc) as tc, ExitStack() as ctx:
        sb = ctx.enter_context(tc.tile_pool(name="sb", bufs=1))
        xp = ctx.enter_context(tc.tile_pool(name="xp", bufs=2))
        ptp = ctx.enter_context(tc.tile_pool(name="ptp", bufs=4))
        cp = ctx.enter_context(tc.tile_pool(name="cp", bufs=4))
        ps = ctx.enter_context(tc.tile_pool(name="ps", bufs=1, space="PSUM"))

        # ---- resident tiles (issue order matters: first matmuls need
        #      only wq + the first xT quarter; wo only at output proj) ----
        xq_tiles = [xp.tile([P, KO, 512], F32R, tag="xq", name=f"xq{i}")
                    for i in range(4)]

        def load_xq(qtr):
            sq = slice(qtr * 512, (qtr + 1) * 512)
            nc.sync.dma_start(xq_tiles[qtr][:, 0:KO // 2, :],
                              xT_r[:, 0:KO // 2, sq])
            nc.sync.dma_start(xq_tiles[qtr][:, KO // 2:KO, :],
                              xT_r[:, KO // 2:KO, sq])

        wq_t = sb.tile([P, KO, W], F32R)
        nc.scalar.dma_start(wq_t[:], wq_r)
        load_xq(0)
        bq_t = sb.tile([P, MT], F32)
        nc.sync.dma_start(bq_t[:], bq)
        wk_t = sb.tile([P, KO, W], F32R)
        nc.scalar.dma_start(wk_t[:], wk_r)
        bk_t = sb.tile([P, MT], F32)
        nc.sync.dma_start(bk_t[:], bk)
        wv_t = sb.tile([P, KO, W], F32R)
        nc.sync.dma_start(wv_t[:], wv_r)
        bv_t = sb.tile([P, W], F32)
        nc.sync.dma_start(bv_t[:], bv)
        wo_t = sb.tile([P, MT, D], F32R)
        nc.sync.dma_start(wo_t[:], wo_r)

        qt_t = sb.tile([P, MT, S], F32R)      # [dk-pair, hp, seq]
        kt_t = sb.tile([P, MT, S], F32R)
        # vaug: per sk chunk, per head h: cols h*128+0:64 = V_h + bv,
        #       cols h*128+64:128 = ones (rowsum trick)
        vaug_t = sb.tile([P, NSK, 4 * P], F32R)
        ones_view = vaug_t[:].rearrange("p s (h c) -> p s h c", c=P)[:, :, :, DK:P]
        nc.vector.memset(ones_view.bitcast(F32), 1.0)
        ot_t = sb.tile([P, MT, S], F32R)      # normalized O^T

        # One (q, hp) attention block as a resumable generator of s-steps so
        # the first block can interleave with the projection quarters.
        _uid = [0]

        def attn_block_steps(q, hp):
            _uid[0] += 1
            u = _uid[0]
            sq = slice(q * 512, (q + 1) * 512)
            oA = ps.tile([P, 512], F32, tag="oA", bufs=1, name=f"oA{u}_{q}_{hp}")
            oB = ps.tile([P, 512], F32, tag="oB", bufs=1, name=f"oB{u}_{q}_{hp}")
            hA, hB = 2 * hp, 2 * hp + 1
            pts = {}

            def attn_v(s):
                pt = pts.pop(s)
                nc.tensor.matmul(oA[:], vaug_t[:, s, hA * P:(hA + 1) * P],
                                 pt[:, 0:512],
                                 start=(s == 0), stop=(s == NSK - 1))
                nc.tensor.matmul(oB[:], vaug_t[:, s, hB * P:(hB + 1) * P],
                                 pt[:, 512:1024],
                                 start=(s == 0), stop=(s == NSK - 1))

            for s in range(NSK):
                sp = ps.tile([P, 1024], F32, tag="spair", bufs=2,
                             name=f"sp{u}_{q}_{hp}_{s}")
                ks = slice(s * P, (s + 1) * P)
                nc.tensor.matmul(sp[:, 0:512], kt_t[0:64, hp, ks],
                                 qt_t[0:64, hp, sq], start=True, stop=True)
                nc.tensor.matmul(sp[:, 512:1024], kt_t[64:P, hp, ks],
                                 qt_t[64:P, hp, sq], start=True, stop=True)
                pt = ptp.tile([P, 1024], F32R, tag="pt", name=f"pt{u}_{q}_{hp}_{s}")
                nc.scalar.activation(pt[:], sp[:],
                                     mybir.ActivationFunctionType.Exp,
                                     bias=0.0, scale=SCALE)
                pts[s] = pt
                if s >= 1:
                    attn_v(s - 1)
                yield s
            attn_v(NSK - 1)
            rsA = cp.tile([64, 512], F32, tag="rs", bufs=2, name=f"rA{u}_{q}_{hp}")
            nc.vector.reciprocal(rsA[:], oA[64:P, :])
            nc.vector.tensor_mul(ot_t[0:64, hp, sq], oA[0:64, :], rsA[:])
            rsB = cp.tile([64, 512], F32, tag="rs", bufs=2, name=f"rB{u}_{q}_{hp}")
            nc.vector.reciprocal(rsB[:], oB[64:P, :])
            nc.vector.tensor_mul(ot_t[64:P, hp, sq], oB[0:64, :], rsB[:])
            yield NSK

        # ---- phase 1: projections, quarter of seq at a time ----
        for _rep in range(repeat):
          first_blk = attn_block_steps(0, 0)
          for qtr in range(4):
              sq = slice(qtr * 512, (qtr + 1) * 512)
              if qtr > 0 or _rep > 0:
                  if _rep > 0:
                      xq_tiles[qtr] = xp.tile([P, KO, 512], F32R, tag="xq",
                                              name=f"xqr{qtr}")
                  load_xq(qtr)
              xq_t = xq_tiles[qtr]
              for dst, wt, bt in ((qt_t, wq_t, bq_t), (kt_t, wk_t, bk_t)):
                  for m in range(MT):
                      pp = ps.tile([P, 512], F32, tag="po", bufs=2)
                      for k in range(KO):
                          nc.tensor.matmul(pp[:], wt[:, k, m * P:(m + 1) * P],
                                           xq_t[:, k, :],
                                           start=(k == 0), stop=(k == KO - 1))
                      nc.vector.tensor_scalar_add(dst[:, m, sq], pp[:], bt[:, m:m + 1])
              for st in range(4):  # V for 4 seq-tiles of 128 within the quarter
                  pv = ps.tile([P, W], F32, tag="po", bufs=2)
                  for k in range(KO):
                      nc.tensor.matmul(pv[:], xq_t[:, k, st * P:(st + 1) * P],
                                       wv_t[:, k, :],
                                       start=(k == 0), stop=(k == KO - 1))
                  skc = qtr * 4 + st
                  vdst = vaug_t[:, skc, :].rearrange("p (h c) -> p h c", c=P)[:, :, 0:DK]
                  nc.vector.tensor_add(
                      vdst,
                      pv[:].rearrange("p (h c) -> p h c", c=DK),
                      bv_t[:].rearrange("p (h c) -> p h c", c=DK))
              for _ in range(4):
                  next(first_blk, None)

          # ---- phase 2+3: attention + output projection ----
          for q in range(NQ):
              for hp in range(MT):
                  if (q, hp) == (0, 0):
                      for _ in first_blk:
                          pass
                  else:
                      for _ in attn_block_steps(q, hp):
                          pass
              # output projection for the 4 seq-tiles covered by this q chunk
              for st in range(4):
                  mo = q * 4 + st
                  for n in range(2):
                      po = ps.tile([P, 512], F32, tag="po", bufs=2)
                      for k in range(MT):
                          nc.tensor.matmul(
                              po[:], ot_t[:, k, mo * P:(mo + 1) * P],
                              wo_t[:, k, n * 512:(n + 1) * 512],
                              start=(k == 0), stop=(k == MT - 1))
                      ob = cp.tile([P, 512], F32, tag="ob")
                      nc.vector.tensor_copy(ob[:], po[:])
                      nc.sync.dma_start(
                          out[mo * P:(mo + 1) * P, n * 512:(n + 1) * 512], ob[:])
    nc.compile()
    return nc


def _prep_inputs(x, Wq, bq, Wk, bk, Wv, bv, Wo, bo):
    in_maps = []
    xTb = [to_f32r(np.ascontiguousarray(x[b].T)) for b in range(B)]
    for c in range(8):
        b, g = c // 4, c % 4
        cs = slice(g * W, (g + 1) * W)
        in_maps.append({
            "xT": xTb[b],
            "wq": to_f32r(Wq[:, cs]),
            "wk": to_f32r(Wk[:, cs]),
            "wv": to_f32r(Wv[:, cs]),
            "wo": to_f32r(Wo[cs, :]),
            "bq": np.ascontiguousarray(bq[cs].reshape(MT, P).T),
            "bk": np.ascontiguousarray(bk[cs].reshape(MT, P).T),
            "bv": np.tile(bv[cs], (P, 1)),
        })
    return in_maps


def kernel(x, Wq, bq, Wk, bk, Wv, bv, Wo, bo):
    x = np.asarray(x, dtype=np.float32)
    Wq, bq = np.asarray(Wq, np.float32), np.asarray(bq, np.float32)
    Wk, bk = np.asarray(Wk, np.float32), np.asarray(bk, np.float32)
    Wv, bv = np.asarray(Wv, np.float32), np.asarray(bv, np.float32)
    Wo, bo = np.asarray(Wo, np.float32), np.asarray(bo, np.float32)

    if "nc" not in _CACHE:
        _CACHE["nc"] = build_nc()
    nc = _CACHE["nc"]

    in_maps = _prep_inputs(x, Wq, bq, Wk, bk, Wv, bv, Wo, bo)
    res = run_bass_kernel_spmd(nc, in_maps, core_ids=list(range(8))).results

    out = np.empty((B, S, D), dtype=np.float32)
    for b in range(B):
        acc = res[4 * b]["out"].copy()
        for g in range(1, 4):
            acc += res[4 * b + g]["out"]
        out[b] = acc + bo
    return out



# revision 31
# speedup vs baseline: 1.0195x; 1.0010x over previous
"""Multi-head attention (B=2, S=2048, D=1024, H=16, Dk=64) on 8 NeuronCores.

Sharding: 2-way data parallel over batch x 4-way tensor parallel over heads.
Core c = 4*b + g handles batch b, head group g (4 heads = 256 cols).
W_o is row-sliced; the 4 partial outputs per batch are summed on host (+bo).

Per-core layout (all transposed — zero on-device transposes):
  host passes xT = x[b].T as float32r (TF32-like, rounded host-side)
  QT/KT = Wg^T @ xT computed as [256, 2048] (dk on partitions)
  scores^T = K @ Q^T per head; softmax along partitions (exp without max —
  scores are O(1) by construction); rowsums via 64 ones-columns appended to
  V in the P@V matmul (M=128: rows 0-63 = O^T unnorm, 64-127 = rowsum x64)
  out_partial = O^T.T @ Wo_rows directly from the O^T layout.
"""
import numpy as np
from contextlib import ExitStack

import concourse.bass as bass
import concourse.mybir as mybir
import concourse.tile as tile
from concourse import bacc
from concourse.bass_utils import run_bass_kernel_spmd

F32 = mybir.dt.float32
F32R = mybir.dt.float32r

B, S, D = 2, 2048, 1024
H, DK = 16, 64
P = 128
KO = D // P          # 8 contraction subtiles for the projections
W = 256              # local width (4 heads x 64)
MT = W // P          # 2 m-tiles = head pairs
NQ = S // 512        # 4 sq chunks
NSK = S // P         # 16 sk chunks
SCALE = 1.0 / 8.0    # 1/sqrt(DK)

_CACHE = {}


def to_f32r(x: np.ndarray) -> np.ndarray:
    """Round fp32 to the fp32r bit layout (8e11m in the top 20 bits, RNE)."""
    u = np.ascontiguousarray(x, dtype=np.float32).view(np.uint32)
    r = u + np.uint32(0x7FF) + ((u >> np.uint32(12)) & np.uint32(1))
    r &= np.uint32(0xFFFFF000)
    return r.view(np.float32)


def build_nc(repeat=1):
    nc = bacc.Bacc("TRN2", target_bir_lowering=False, debug=False, num_devices=8)
    xT = nc.dram_tensor("xT", [D, S], F32R, kind="ExternalInput").ap()
    wq = nc.dram_tensor("wq", [D, W], F32R, kind="ExternalInput").ap()
    wk = nc.dram_tensor("wk", [D, W], F32R, kind="ExternalInput").ap()
    wv = nc.dram_tensor("wv", [D, W], F32R, kind="ExternalInput").ap()
    wo = nc.dram_tensor("wo", [W, D], F32R, kind="ExternalInput").ap()
    bq = nc.dram_tensor("bq", [P, MT], F32, kind="ExternalInput").ap()
    bk = nc.dram_tensor("bk", [P, MT], F32, kind="ExternalInput").ap()
    bv = nc.dram_tensor("bv", [P, W], F32, kind="ExternalInput").ap()
    out = nc.dram_tensor("out", [S, D], F32, kind="ExternalOutput").ap()

    xT_r = xT.rearrange("(ko p) s -> p ko s", p=P)
    wq_r = wq.rearrange("(ko p) w -> p ko w", p=P)
    wk_r = wk.rearrange("(ko p) w -> p ko w", p=P)
    wv_r = wv.rearrange("(ko p) w -> p ko w", p=P)
    wo_r = wo.rearrange("(ko p) d -> p ko d", p=P)

    with tile.TileContext(nc) as tc, ExitStack() as ctx:
        sb = ctx.enter_context(tc.tile_pool(name="sb", bufs=1))
        xp = ctx.enter_context(tc.tile_pool(name="xp", bufs=2))
        ptp = ctx.enter_context(tc.tile_pool(name="ptp", bufs=4))
        cp = ctx.enter_context(tc.tile_pool(name="cp", bufs=4))
        ps = ctx.enter_context(tc.tile_pool(name="ps", bufs=1, space="PSUM"))

        # ---- resident tiles (issue order matters: first matmuls need
        #      only wq + the first xT quarter; wo only at output proj) ----
        xq_tiles = [xp.tile([P, KO, 512], F32R, tag="xq", name=f"xq{i}")
                    for i in range(4)]

        def load_xq(qtr):
            sq = slice(qtr * 512, (qtr + 1) * 512)
            nc.sync.dma_start(xq_tiles[qtr][:, 0:KO // 2, :],
                              xT_r[:, 0:KO // 2, sq])
            nc.sync.dma_start(xq_tiles[qtr][:, KO // 2:KO, :],
                              xT_r[:, KO // 2:KO, sq])

        wq_t = sb.tile([P, KO, W], F32R)
        nc.scalar.dma_start(wq_t[:], wq_r)
        load_xq(0)
        bq_t = sb.tile([P, MT], F32)
        nc.sync.dma_start(bq_t[:], bq)
        wk_t = sb.tile([P, KO, W], F32R)
        nc.scalar.dma_start(wk_t[:], wk_r)
        bk_t = sb.tile([P, MT], F32)
        nc.sync.dma_start(bk_t[:], bk)
        wv_t = sb.tile([P, KO, W], F32R)
        nc.sync.dma_start(wv_t[:], wv_r)
        bv_t = sb.tile([P, W], F32)
        nc.sync.dma_start(bv_t[:], bv)
        wo_t = sb.tile([P, MT, D], F32R)
        nc.sync.dma_start(wo_t[:], wo_r)

        qt_t = sb.tile([P, MT, S], F32R)      # [dk-pair, hp, seq]
        kt_t = sb.tile([P, MT, S], F32R)
        # vaug: per sk chunk, per head h: cols h*128+0:64 = V_h + bv,
        #       cols h*128+64:128 = ones (rowsum trick)
        vaug_t = sb.tile([P, NSK, 4 * P], F32R)
        ones_view = vaug_t[:].rearrange("p s (h c) -> p s h c", c=P)[:, :, :, DK:P]
        nc.vector.memset(ones_view.bitcast(F32), 1.0)
        ot_t = sb.tile([P, MT, S], F32R)      # normalized O^T

        # One (q, hp) attention block as a resumable generator of s-steps so
        # the first block can interleave with the projection quarters.
        _uid = [0]

        def attn_block_steps(q, hp):
            _uid[0] += 1
            u = _uid[0]
            sq = slice(q * 512, (q + 1) * 512)
            oA = ps.tile([P, 512], F32, tag="oA", bufs=1, name=f"oA{u}_{q}_{hp}")
            oB = ps.tile([P, 512], F32, tag="oB", bufs=1, name=f"oB{u}_{q}_{hp}")
            hA, hB = 2 * hp, 2 * hp + 1
            pts = {}

            def attn_v(s):
                pt = pts.pop(s)
                nc.tensor.matmul(oA[:], vaug_t[:, s, hA * P:(hA + 1) * P],
                                 pt[:, 0:512],
                                 start=(s == 0), stop=(s == NSK - 1))
                nc.tensor.matmul(oB[:], vaug_t[:, s, hB * P:(hB + 1) * P],
                                 pt[:, 512:1024],
                                 start=(s == 0), stop=(s == NSK - 1))

            for s in range(NSK):
                sp = ps.tile([P, 1024], F32, tag="spair", bufs=2,
                             name=f"sp{u}_{q}_{hp}_{s}")
                ks = slice(s * P, (s + 1) * P)
                nc.tensor.matmul(sp[:, 0:512], kt_t[0:64, hp, ks],
                                 qt_t[0:64, hp, sq], start=True, stop=True)
                nc.tensor.matmul(sp[:, 512:1024], kt_t[64:P, hp, ks],
                                 qt_t[64:P, hp, sq], start=True, stop=True)
                pt = ptp.tile([P, 1024], F32R, tag="pt", name=f"pt{u}_{q}_{hp}_{s}")
                nc.scalar.activation(pt[:], sp[:],
                                     mybir.ActivationFunctionType.Exp,
                                     bias=0.0, scale=SCALE)
                pts[s] = pt
                if s >= 1:
                    attn_v(s - 1)
                yield s
            attn_v(NSK - 1)
            rsA = cp.tile([64, 512], F32, tag="rs", bufs=2, name=f"rA{u}_{q}_{hp}")
            nc.vector.reciprocal(rsA[:], oA[64:P, :])
            nc.vector.tensor_mul(ot_t[0:64, hp, sq], oA[0:64, :], rsA[:])
            rsB = cp.tile([64, 512], F32, tag="rs", bufs=2, name=f"rB{u}_{q}_{hp}")
            nc.vector.reciprocal(rsB[:], oB[64:P, :])
            nc.vector.tensor_mul(ot_t[64:P, hp, sq], oB[0:64, :], rsB[:])
            yield NSK

        # ---- phase 1: projections, quarter of seq at a time ----
        for _rep in range(repeat):
          first_blk = attn_block_steps(0, 0)
          for qtr in range(4):
              sq = slice(qtr * 512, (qtr + 1) * 512)
              if qtr > 0 or _rep > 0:
                  if _rep > 0:
                      xq_tiles[qtr] = xp.tile([P, KO, 512], F32R, tag="xq",
                                              name=f"xqr{qtr}")
                  load_xq(qtr)
              xq_t = xq_tiles[qtr]
              for dst, wt, bt in ((qt_t, wq_t, bq_t), (kt_t, wk_t, bk_t)):
                  for m in range(MT):
                      pp = ps.tile([P, 512], F32, tag="po", bufs=2)
                      for k in range(KO):
                          nc.tensor.matmul(pp[:], wt[:, k, m * P:(m + 1) * P],
                                           xq_t[:, k, :],
                                           start=(k == 0), stop=(k == KO - 1))
                      nc.vector.tensor_scalar_add(dst[:, m, sq], pp[:], bt[:, m:m + 1])
              # step 4*qtr needs only this quarter's QT/KT (its attn_v
              # touches the previous quarter's vaug) — advance it before the
              # V projections so ACT starts its exp ~3.4us earlier
              next(first_blk, None)
              for st in range(4):  # V for 4 seq-tiles of 128 within the quarter
                  pv = ps.tile([P, W], F32, tag="po", bufs=2)
                  for k in range(KO):
                      nc.tensor.matmul(pv[:], xq_t[:, k, st * P:(st + 1) * P],
                                       wv_t[:, k, :],
                                       start=(k == 0), stop=(k == KO - 1))
                  skc = qtr * 4 + st
                  vdst = vaug_t[:, skc, :].rearrange("p (h c) -> p h c", c=P)[:, :, 0:DK]
                  nc.vector.tensor_add(
                      vdst,
                      pv[:].rearrange("p (h c) -> p h c", c=DK),
                      bv_t[:].rearrange("p (h c) -> p h c", c=DK))
              for _ in range(3):
                  next(first_blk, None)

          # ---- phase 2+3: attention + output projection ----
          for q in range(NQ):
              for hp in range(MT):
                  if (q, hp) == (0, 0):
                      for _ in first_blk:
                          pass
                  else:
                      for _ in attn_block_steps(q, hp):
                          pass
              # output projection for the 4 seq-tiles covered by this q chunk
              for st in range(4):
                  mo = q * 4 + st
                  for n in range(2):
                      po = ps.tile([P, 512], F32, tag="po", bufs=2)
                      for k in range(MT):
                          nc.tensor.matmul(
                              po[:], ot_t[:, k, mo * P:(mo + 1) * P],
                              wo_t[:, k, n * 512:(n + 1) * 512],
                              start=(k == 0), stop=(k == MT - 1))
                      ob = cp.tile([P, 512], F32, tag="ob")
                      nc.vector.tensor_copy(ob[:], po[:])
                      nc.sync.dma_start(
                          out[mo * P:(mo + 1) * P, n * 512:(n + 1) * 512], ob[:])
    nc.compile()
    return nc


def _prep_inputs(x, Wq, bq, Wk, bk, Wv, bv, Wo, bo):
    in_maps = []
    xTb = [to_f32r(np.ascontiguousarray(x[b].T)) for b in range(B)]
    for c in range(8):
        b, g = c // 4, c % 4
        cs = slice(g * W, (g + 1) * W)
        in_maps.append({
            "xT": xTb[b],
            "wq": to_f32r(Wq[:, cs]),
            "wk": to_f32r(Wk[:, cs]),
            "wv": to_f32r(Wv[:, cs]),
            "wo": to_f32r(Wo[cs, :]),
            "bq": np.ascontiguousarray(bq[cs].reshape(MT, P).T),
            "bk": np.ascontiguousarray(bk[cs].reshape(MT, P).T),
            "bv": np.tile(bv[cs], (P, 1)),
        })
    return in_maps


def kernel(x, Wq, bq, Wk, bk, Wv, bv, Wo, bo):
    x = np.asarray(x, dtype=np.float32)
    Wq, bq = np.asarray(Wq, np.float32), np.asarray(bq, np.float32)
    Wk, bk = np.asarray(Wk, np.float32), np.asarray(bk, np.float32)
    Wv, bv = np.asarray(Wv, np.float32), np.asarray(bv, np.float32)
    Wo, bo = np.asarray(Wo, np.float32), np.asarray(bo, np.float32)

    if "nc" not in _CACHE:
        _CACHE["nc"] = build_nc()
    nc = _CACHE["nc"]

    in_maps = _prep_inputs(x, Wq, bq, Wk, bk, Wv, bv, Wo, bo)
    res = run_bass_kernel_spmd(nc, in_maps, core_ids=list(range(8))).results

    out = np.empty((B, S, D), dtype=np.float32)
    for b in range(B):
        acc = res[4 * b]["out"].copy()
        for g in range(1, 4):
            acc += res[4 * b + g]["out"]
        out[b] = acc + bo
    return out



# revision 36
# speedup vs baseline: 1.0300x; 1.0103x over previous
"""Multi-head attention (B=2, S=2048, D=1024, H=16, Dk=64) on 8 NeuronCores.

Sharding: 2-way data parallel over batch x 4-way tensor parallel over heads.
Core c = 4*b + g handles batch b, head group g (4 heads = 256 cols).
W_o is row-sliced; the 4 partial outputs per batch are summed on host (+bo).

Per-core layout (all transposed — zero on-device transposes):
  host passes xT = x[b].T as float32r (TF32-like, rounded host-side)
  QT/KT = Wg^T @ xT computed as [256, 2048] (dk on partitions)
  scores^T = K @ Q^T per head; softmax along partitions (exp without max —
  scores are O(1) by construction); rowsums via 64 ones-columns appended to
  V in the P@V matmul (M=128: rows 0-63 = O^T unnorm, 64-127 = rowsum x64)
  out_partial = O^T.T @ Wo_rows directly from the O^T layout.
"""
import numpy as np
from contextlib import ExitStack

import concourse.bass as bass
import concourse.mybir as mybir
import concourse.tile as tile
from concourse import bacc
from concourse.bass_utils import run_bass_kernel_spmd

F32 = mybir.dt.float32
F32R = mybir.dt.float32r

B, S, D = 2, 2048, 1024
H, DK = 16, 64
P = 128
KO = D // P          # 8 contraction subtiles for the projections
W = 256              # local width (4 heads x 64)
MT = W // P          # 2 m-tiles = head pairs
NQ = S // 512        # 4 sq chunks
NSK = S // P         # 16 sk chunks
SCALE = 1.0 / 8.0    # 1/sqrt(DK)

_CACHE = {}


def to_f32r(x: np.ndarray) -> np.ndarray:
    """Round fp32 to the fp32r bit layout (8e11m in the top 20 bits, RNE)."""
    u = np.ascontiguousarray(x, dtype=np.float32).view(np.uint32)
    r = u + np.uint32(0x7FF) + ((u >> np.uint32(12)) & np.uint32(1))
    r &= np.uint32(0xFFFFF000)
    return r.view(np.float32)


def build_nc(repeat=1):
    nc = bacc.Bacc("TRN2", target_bir_lowering=False, debug=False, num_devices=8)
    xT = nc.dram_tensor("xT", [D, S], F32R, kind="ExternalInput").ap()
    wq = nc.dram_tensor("wq", [D, W], F32R, kind="ExternalInput").ap()
    wk = nc.dram_tensor("wk", [D, W], F32R, kind="ExternalInput").ap()
    wv = nc.dram_tensor("wv", [D, W], F32R, kind="ExternalInput").ap()
    wo = nc.dram_tensor("wo", [W, D], F32R, kind="ExternalInput").ap()
    bq = nc.dram_tensor("bq", [P, MT], F32, kind="ExternalInput").ap()
    bk = nc.dram_tensor("bk", [P, MT], F32, kind="ExternalInput").ap()
    bv = nc.dram_tensor("bv", [P, W], F32, kind="ExternalInput").ap()
    out = nc.dram_tensor("out", [S, D], F32, kind="ExternalOutput").ap()

    xT_r = xT.rearrange("(ko p) s -> p ko s", p=P)
    wq_r = wq.rearrange("(ko p) w -> p ko w", p=P)
    wk_r = wk.rearrange("(ko p) w -> p ko w", p=P)
    wv_r = wv.rearrange("(ko p) w -> p ko w", p=P)
    wo_r = wo.rearrange("(ko p) d -> p ko d", p=P)

    with tile.TileContext(nc) as tc, ExitStack() as ctx:
        sb = ctx.enter_context(tc.tile_pool(name="sb", bufs=1))
        xp = ctx.enter_context(tc.tile_pool(name="xp", bufs=2))
        ptp = ctx.enter_context(tc.tile_pool(name="ptp", bufs=4))
        cp = ctx.enter_context(tc.tile_pool(name="cp", bufs=8))
        ps = ctx.enter_context(tc.tile_pool(name="ps", bufs=1, space="PSUM"))

        # ---- resident tiles (issue order matters: first matmuls need
        #      only wq + the first xT quarter; wo only at output proj) ----
        xq_tiles = [xp.tile([P, KO, 512], F32R, tag="xq", name=f"xq{i}")
                    for i in range(4)]

        def load_xq(qtr):
            sq = slice(qtr * 512, (qtr + 1) * 512)
            nc.sync.dma_start(xq_tiles[qtr][:, 0:KO // 2, :],
                              xT_r[:, 0:KO // 2, sq])
            nc.sync.dma_start(xq_tiles[qtr][:, KO // 2:KO, :],
                              xT_r[:, KO // 2:KO, sq])

        wq_t = sb.tile([P, KO, W], F32R)
        nc.scalar.dma_start(wq_t[:], wq_r)
        load_xq(0)
        bq_t = sb.tile([P, MT], F32)
        nc.sync.dma_start(bq_t[:], bq)
        wk_t = sb.tile([P, KO, W], F32R)
        nc.scalar.dma_start(wk_t[:], wk_r)
        bk_t = sb.tile([P, MT], F32)
        nc.sync.dma_start(bk_t[:], bk)
        wv_t = sb.tile([P, KO, W], F32R)
        nc.sync.dma_start(wv_t[:], wv_r)
        bv_t = sb.tile([P, W], F32)
        nc.sync.dma_start(bv_t[:], bv)
        wo_t = sb.tile([P, MT, D], F32R)
        nc.sync.dma_start(wo_t[:], wo_r)

        qt_t = sb.tile([P, MT, S], F32R)      # [dk-pair, hp, seq]
        kt_t = sb.tile([P, MT, S], F32R)
        # vaug: per sk chunk, per head h: cols h*128+0:64 = V_h + bv,
        #       cols h*128+64:128 = ones (rowsum trick)
        vaug_t = sb.tile([P, NSK, 4 * P], F32R)
        ones_view = vaug_t[:].rearrange("p s (h c) -> p s h c", c=P)[:, :, :, DK:P]
        nc.vector.memset(ones_view.bitcast(F32), 1.0)
        ot_t = sb.tile([P, MT, S], F32R)      # normalized O^T

        # One (q, hp) attention block as a resumable generator of s-steps so
        # the first block can interleave with the projection quarters.
        _uid = [0]

        def attn_block_steps(q, hp):
            _uid[0] += 1
            u = _uid[0]
            sq = slice(q * 512, (q + 1) * 512)
            oA = ps.tile([P, 512], F32, tag="oA", bufs=1, name=f"oA{u}_{q}_{hp}")
            oB = ps.tile([P, 512], F32, tag="oB", bufs=1, name=f"oB{u}_{q}_{hp}")
            hA, hB = 2 * hp, 2 * hp + 1
            pts = {}

            def attn_v(s):
                pt = pts.pop(s)
                nc.tensor.matmul(oA[:], vaug_t[:, s, hA * P:(hA + 1) * P],
                                 pt[:, 0:512],
                                 start=(s == 0), stop=(s == NSK - 1))
                nc.tensor.matmul(oB[:], vaug_t[:, s, hB * P:(hB + 1) * P],
                                 pt[:, 512:1024],
                                 start=(s == 0), stop=(s == NSK - 1))

            for s in range(NSK):
                sp = ps.tile([P, 1024], F32, tag="spair", bufs=2,
                             name=f"sp{u}_{q}_{hp}_{s}")
                ks = slice(s * P, (s + 1) * P)
                nc.tensor.matmul(sp[:, 0:512], kt_t[0:64, hp, ks],
                                 qt_t[0:64, hp, sq], start=True, stop=True)
                nc.tensor.matmul(sp[:, 512:1024], kt_t[64:P, hp, ks],
                                 qt_t[64:P, hp, sq], start=True, stop=True)
                pt = ptp.tile([P, 1024], F32R, tag="pt", name=f"pt{u}_{q}_{hp}_{s}")
                nc.scalar.activation(pt[:], sp[:],
                                     mybir.ActivationFunctionType.Exp,
                                     bias=0.0, scale=SCALE)
                pts[s] = pt
                if s >= 1:
                    attn_v(s - 1)
                yield s
            attn_v(NSK - 1)
            rsA = cp.tile([64, 512], F32, tag="rs", bufs=2, name=f"rA{u}_{q}_{hp}")
            nc.vector.reciprocal(rsA[:], oA[64:P, :])
            nc.vector.tensor_mul(ot_t[0:64, hp, sq], oA[0:64, :], rsA[:])
            rsB = cp.tile([64, 512], F32, tag="rs", bufs=2, name=f"rB{u}_{q}_{hp}")
            nc.vector.reciprocal(rsB[:], oB[64:P, :])
            nc.vector.tensor_mul(ot_t[64:P, hp, sq], oB[0:64, :], rsB[:])
            yield NSK

        # ---- phase 1: projections, quarter of seq at a time ----
        for _rep in range(repeat):
          first_blk = attn_block_steps(0, 0)
          for qtr in range(4):
              sq = slice(qtr * 512, (qtr + 1) * 512)
              if qtr > 0 or _rep > 0:
                  if _rep > 0:
                      xq_tiles[qtr] = xp.tile([P, KO, 512], F32R, tag="xq",
                                              name=f"xqr{qtr}")
                  load_xq(qtr)
              xq_t = xq_tiles[qtr]
              for dst, wt, bt in ((qt_t, wq_t, bq_t), (kt_t, wk_t, bk_t)):
                  for m in range(MT):
                      pp = ps.tile([P, 512], F32, tag="po", bufs=2)
                      for k in range(KO):
                          nc.tensor.matmul(pp[:], wt[:, k, m * P:(m + 1) * P],
                                           xq_t[:, k, :],
                                           start=(k == 0), stop=(k == KO - 1))
                      nc.vector.tensor_scalar_add(dst[:, m, sq], pp[:], bt[:, m:m + 1])
              # step 4*qtr needs only this quarter's QT/KT (its attn_v
              # touches the previous quarter's vaug) — advance it before the
              # V projections so ACT starts its exp ~3.4us earlier
              next(first_blk, None)
              for st in range(4):  # V for 4 seq-tiles of 128 within the quarter
                  pv = ps.tile([P, W], F32, tag="po", bufs=2)
                  for k in range(KO):
                      nc.tensor.matmul(pv[:], xq_t[:, k, st * P:(st + 1) * P],
                                       wv_t[:, k, :],
                                       start=(k == 0), stop=(k == KO - 1))
                  skc = qtr * 4 + st
                  vdst = vaug_t[:, skc, :].rearrange("p (h c) -> p h c", c=P)[:, :, 0:DK]
                  nc.vector.tensor_add(
                      vdst,
                      pv[:].rearrange("p (h c) -> p h c", c=DK),
                      bv_t[:].rearrange("p (h c) -> p h c", c=DK))
              for _ in range(3):
                  next(first_blk, None)

          # ---- phase 2+3: attention + output projection ----
          for q in range(NQ):
              for hp in range(MT):
                  if (q, hp) == (0, 0):
                      for _ in first_blk:
                          pass
                  else:
                      for _ in attn_block_steps(q, hp):
                          pass
              # output projection for the 4 seq-tiles covered by this q chunk
              for st in range(4):
                  mo = q * 4 + st
                  for n in range(2):
                      po = ps.tile([P, 512], F32, tag="po", bufs=2)
                      for k in range(MT):
                          nc.tensor.matmul(
                              po[:], ot_t[:, k, mo * P:(mo + 1) * P],
                              wo_t[:, k, n * 512:(n + 1) * 512],
                              start=(k == 0), stop=(k == MT - 1))
                      ob = cp.tile([P, 512], F32, tag="ob")
                      nc.vector.tensor_copy(ob[:], po[:])
                      nc.sync.dma_start(
                          out[mo * P:(mo + 1) * P, n * 512:(n + 1) * 512], ob[:])
    nc.compile()
    return nc


def _prep_inputs(x, Wq, bq, Wk, bk, Wv, bv, Wo, bo):
    in_maps = []
    xTb = [to_f32r(np.ascontiguousarray(x[b].T)) for b in range(B)]
    for c in range(8):
        b, g = c // 4, c % 4
        cs = slice(g * W, (g + 1) * W)
        in_maps.append({
            "xT": xTb[b],
            "wq": to_f32r(Wq[:, cs]),
            "wk": to_f32r(Wk[:, cs]),
            "wv": to_f32r(Wv[:, cs]),
            "wo": to_f32r(Wo[cs, :]),
            "bq": np.ascontiguousarray(bq[cs].reshape(MT, P).T),
            "bk": np.ascontiguousarray(bk[cs].reshape(MT, P).T),
            "bv": np.tile(bv[cs], (P, 1)),
        })
    return in_maps


def kernel(x, Wq, bq, Wk, bk, Wv, bv, Wo, bo):
    x = np.asarray(x, dtype=np.float32)
    Wq, bq = np.asarray(Wq, np.float32), np.asarray(bq, np.float32)
    Wk, bk = np.asarray(Wk, np.float32), np.asarray(bk, np.float32)
    Wv, bv = np.asarray(Wv, np.float32), np.asarray(bv, np.float32)
    Wo, bo = np.asarray(Wo, np.float32), np.asarray(bo, np.float32)

    if "nc" not in _CACHE:
        _CACHE["nc"] = build_nc()
    nc = _CACHE["nc"]

    in_maps = _prep_inputs(x, Wq, bq, Wk, bk, Wv, bv, Wo, bo)
    res = run_bass_kernel_spmd(nc, in_maps, core_ids=list(range(8))).results

    out = np.empty((B, S, D), dtype=np.float32)
    for b in range(B):
        acc = res[4 * b]["out"].copy()
        for g in range(1, 4):
            acc += res[4 * b + g]["out"]
        out[b] = acc + bo
    return out



# revision 40
# speedup vs baseline: 1.0343x; 1.0042x over previous
"""Multi-head attention (B=2, S=2048, D=1024, H=16, Dk=64) on 8 NeuronCores.

Sharding: 2-way data parallel over batch x 4-way tensor parallel over heads.
Core c = 4*b + g handles batch b, head group g (4 heads = 256 cols).
W_o is row-sliced; the 4 partial outputs per batch are summed on host (+bo).

Per-core layout (all transposed — zero on-device transposes):
  host passes xT = x[b].T as float32r (TF32-like, rounded host-side)
  QT/KT = Wg^T @ xT computed as [256, 2048] (dk on partitions)
  scores^T = K @ Q^T per head; softmax along partitions (exp without max —
  scores are O(1) by construction); rowsums via 64 ones-columns appended to
  V in the P@V matmul (M=128: rows 0-63 = O^T unnorm, 64-127 = rowsum x64)
  out_partial = O^T.T @ Wo_rows directly from the O^T layout.
"""
import numpy as np
from contextlib import ExitStack

import concourse.bass as bass
import concourse.mybir as mybir
import concourse.tile as tile
from concourse import bacc
from concourse.bass_utils import run_bass_kernel_spmd

F32 = mybir.dt.float32
F32R = mybir.dt.float32r

B, S, D = 2, 2048, 1024
H, DK = 16, 64
P = 128
KO = D // P          # 8 contraction subtiles for the projections
W = 256              # local width (4 heads x 64)
MT = W // P          # 2 m-tiles = head pairs
NQ = S // 512        # 4 sq chunks
NSK = S // P         # 16 sk chunks
SCALE = 1.0 / 8.0    # 1/sqrt(DK)

_CACHE = {}


def to_f32r(x: np.ndarray) -> np.ndarray:
    """Round fp32 to the fp32r bit layout (8e11m in the top 20 bits, RNE)."""
    u = np.ascontiguousarray(x, dtype=np.float32).view(np.uint32)
    r = u + np.uint32(0x7FF) + ((u >> np.uint32(12)) & np.uint32(1))
    r &= np.uint32(0xFFFFF000)
    return r.view(np.float32)


def build_nc(repeat=1):
    nc = bacc.Bacc("TRN2", target_bir_lowering=False, debug=False, num_devices=8)
    xT = nc.dram_tensor("xT", [D, S], F32R, kind="ExternalInput").ap()
    wq = nc.dram_tensor("wq", [D, W], F32R, kind="ExternalInput").ap()
    wk = nc.dram_tensor("wk", [D, W], F32R, kind="ExternalInput").ap()
    wv = nc.dram_tensor("wv", [D, W], F32R, kind="ExternalInput").ap()
    wo = nc.dram_tensor("wo", [W, D], F32R, kind="ExternalInput").ap()
    bq = nc.dram_tensor("bq", [P, MT], F32, kind="ExternalInput").ap()
    bk = nc.dram_tensor("bk", [P, MT], F32, kind="ExternalInput").ap()
    bv = nc.dram_tensor("bv", [P, W], F32, kind="ExternalInput").ap()
    out = nc.dram_tensor("out", [S, D], F32, kind="ExternalOutput").ap()

    xT_r = xT.rearrange("(ko p) s -> p ko s", p=P)
    wq_r = wq.rearrange("(ko p) w -> p ko w", p=P)
    wk_r = wk.rearrange("(ko p) w -> p ko w", p=P)
    wv_r = wv.rearrange("(ko p) w -> p ko w", p=P)
    wo_r = wo.rearrange("(ko p) d -> p ko d", p=P)

    with tile.TileContext(nc) as tc, ExitStack() as ctx:
        sb = ctx.enter_context(tc.tile_pool(name="sb", bufs=1))
        xp = ctx.enter_context(tc.tile_pool(name="xp", bufs=2))
        ptp = ctx.enter_context(tc.tile_pool(name="ptp", bufs=4))
        cp = ctx.enter_context(tc.tile_pool(name="cp", bufs=8))
        ps = ctx.enter_context(tc.tile_pool(name="ps", bufs=1, space="PSUM"))

        # ---- resident tiles (issue order matters: first matmuls need
        #      only wq + the first xT quarter; wo only at output proj) ----
        xq_tiles = [xp.tile([P, KO, 512], F32R, tag="xq", name=f"xq{i}")
                    for i in range(4)]

        def load_xq(qtr):
            sq = slice(qtr * 512, (qtr + 1) * 512)
            nc.sync.dma_start(xq_tiles[qtr][:, 0:KO // 2, :],
                              xT_r[:, 0:KO // 2, sq])
            nc.sync.dma_start(xq_tiles[qtr][:, KO // 2:KO, :],
                              xT_r[:, KO // 2:KO, sq])

        wq_t = sb.tile([P, KO, W], F32R)
        nc.scalar.dma_start(wq_t[:], wq_r)
        load_xq(0)
        bq_t = sb.tile([P, MT], F32)
        nc.sync.dma_start(bq_t[:], bq)
        wk_t = sb.tile([P, KO, W], F32R)
        nc.scalar.dma_start(wk_t[:], wk_r)
        bk_t = sb.tile([P, MT], F32)
        nc.sync.dma_start(bk_t[:], bk)
        wv_t = sb.tile([P, KO, W], F32R)
        nc.sync.dma_start(wv_t[:], wv_r)
        bv_t = sb.tile([P, W], F32)
        nc.sync.dma_start(bv_t[:], bv)
        wo_t = sb.tile([P, MT, D], F32R)
        nc.sync.dma_start(wo_t[:], wo_r)

        qt_t = sb.tile([P, MT, S], F32R)      # [dk-pair, hp, seq]
        kt_t = sb.tile([P, MT, S], F32R)
        # vaug: per sk chunk, per head h: cols h*128+0:64 = V_h + bv,
        #       cols h*128+64:128 = ones (rowsum trick)
        vaug_t = sb.tile([P, NSK, 4 * P], F32R)
        ones_view = vaug_t[:].rearrange("p s (h c) -> p s h c", c=P)[:, :, :, DK:P]
        nc.vector.memset(ones_view.bitcast(F32), 1.0)
        ot_t = sb.tile([P, MT, S], F32R)      # normalized O^T

        # One (q, hp) attention block as a resumable generator of s-steps so
        # the first block can interleave with the projection quarters.
        _uid = [0]

        def attn_block_steps(q, hp):
            _uid[0] += 1
            u = _uid[0]
            sq = slice(q * 512, (q + 1) * 512)
            oA = ps.tile([P, 512], F32, tag="oA", bufs=1, name=f"oA{u}_{q}_{hp}")
            oB = ps.tile([P, 512], F32, tag="oB", bufs=1, name=f"oB{u}_{q}_{hp}")
            hA, hB = 2 * hp, 2 * hp + 1
            pts = {}

            def attn_v(s):
                pt = pts.pop(s)
                nc.tensor.matmul(oA[:], vaug_t[:, s, hA * P:(hA + 1) * P],
                                 pt[:, 0:512],
                                 start=(s == 0), stop=(s == NSK - 1))
                nc.tensor.matmul(oB[:], vaug_t[:, s, hB * P:(hB + 1) * P],
                                 pt[:, 512:1024],
                                 start=(s == 0), stop=(s == NSK - 1))

            for s in range(NSK):
                sp = ps.tile([P, 1024], F32, tag="spair", bufs=2,
                             name=f"sp{u}_{q}_{hp}_{s}")
                ks = slice(s * P, (s + 1) * P)
                nc.tensor.matmul(sp[:, 0:512], kt_t[0:64, hp, ks],
                                 qt_t[0:64, hp, sq], start=True, stop=True)
                nc.tensor.matmul(sp[:, 512:1024], kt_t[64:P, hp, ks],
                                 qt_t[64:P, hp, sq], start=True, stop=True)
                pt = ptp.tile([P, 1024], F32R, tag="pt", name=f"pt{u}_{q}_{hp}_{s}")
                nc.scalar.activation(pt[:], sp[:],
                                     mybir.ActivationFunctionType.Exp,
                                     bias=0.0, scale=SCALE)
                pts[s] = pt
                if s >= 1:
                    attn_v(s - 1)
                yield s
            attn_v(NSK - 1)
            rsA = cp.tile([64, 512], F32, tag="rs", bufs=2, name=f"rA{u}_{q}_{hp}")
            nc.vector.reciprocal(rsA[:], oA[64:P, :])
            nc.vector.tensor_mul(ot_t[0:64, hp, sq], oA[0:64, :], rsA[:])
            rsB = cp.tile([64, 512], F32, tag="rs", bufs=2, name=f"rB{u}_{q}_{hp}")
            nc.vector.reciprocal(rsB[:], oB[64:P, :])
            nc.vector.tensor_mul(ot_t[64:P, hp, sq], oB[0:64, :], rsB[:])
            yield NSK

        # ---- phase 1: projections, quarter of seq at a time ----
        for _rep in range(repeat):
          first_blk = attn_block_steps(0, 0)
          for qtr in range(4):
              sq = slice(qtr * 512, (qtr + 1) * 512)
              if qtr > 0 or _rep > 0:
                  if _rep > 0:
                      xq_tiles[qtr] = xp.tile([P, KO, 512], F32R, tag="xq",
                                              name=f"xqr{qtr}")
                  load_xq(qtr)
              xq_t = xq_tiles[qtr]
              # m-major order: head pair hp lives entirely in m-tile hp of
              # qt/kt, so finishing QT(m0)+KT(m0) first unblocks the
              # interleaved first-block scores one psum-group earlier
              for m in range(MT):
                  for dst, wt, bt in ((qt_t, wq_t, bq_t), (kt_t, wk_t, bk_t)):
                      pp = ps.tile([P, 512], F32, tag="po", bufs=2)
                      for k in range(KO):
                          nc.tensor.matmul(pp[:], wt[:, k, m * P:(m + 1) * P],
                                           xq_t[:, k, :],
                                           start=(k == 0), stop=(k == KO - 1))
                      nc.vector.tensor_scalar_add(dst[:, m, sq], pp[:], bt[:, m:m + 1])
              # step 4*qtr needs only this quarter's QT/KT (its attn_v
              # touches the previous quarter's vaug) — advance it before the
              # V projections so ACT starts its exp ~3.4us earlier
              next(first_blk, None)
              for st in range(4):  # V for 4 seq-tiles of 128 within the quarter
                  pv = ps.tile([P, W], F32, tag="po", bufs=2)
                  for k in range(KO):
                      nc.tensor.matmul(pv[:], xq_t[:, k, st * P:(st + 1) * P],
                                       wv_t[:, k, :],
                                       start=(k == 0), stop=(k == KO - 1))
                  skc = qtr * 4 + st
                  vdst = vaug_t[:, skc, :].rearrange("p (h c) -> p h c", c=P)[:, :, 0:DK]
                  nc.vector.tensor_add(
                      vdst,
                      pv[:].rearrange("p (h c) -> p h c", c=DK),
                      bv_t[:].rearrange("p (h c) -> p h c", c=DK))
                  if st < 3:
                      next(first_blk, None)

          # ---- phase 2+3: attention + output projection ----
          for q in range(NQ):
              for hp in range(MT):
                  if (q, hp) == (0, 0):
                      for _ in first_blk:
                          pass
                  else:
                      for _ in attn_block_steps(q, hp):
                          pass
              # output projection for the 4 seq-tiles covered by this q chunk
              for st in range(4):
                  mo = q * 4 + st
                  for n in range(2):
                      po = ps.tile([P, 512], F32, tag="po", bufs=2)
                      for k in range(MT):
                          nc.tensor.matmul(
                              po[:], ot_t[:, k, mo * P:(mo + 1) * P],
                              wo_t[:, k, n * 512:(n + 1) * 512],
                              start=(k == 0), stop=(k == MT - 1))
                      ob = cp.tile([P, 512], F32, tag="ob")
                      nc.vector.tensor_copy(ob[:], po[:])
                      nc.sync.dma_start(
                          out[mo * P:(mo + 1) * P, n * 512:(n + 1) * 512], ob[:])
    nc.compile()
    return nc


def _prep_inputs(x, Wq, bq, Wk, bk, Wv, bv, Wo, bo):
    in_maps = []
    xTb = [to_f32r(np.ascontiguousarray(x[b].T)) for b in range(B)]
    for c in range(8):
        b, g = c // 4, c % 4
        cs = slice(g * W, (g + 1) * W)
        in_maps.append({
            "xT": xTb[b],
            "wq": to_f32r(Wq[:, cs]),
            "wk": to_f32r(Wk[:, cs]),
            "wv": to_f32r(Wv[:, cs]),
            "wo": to_f32r(Wo[cs, :]),
            "bq": np.ascontiguousarray(bq[cs].reshape(MT, P).T),
            "bk": np.ascontiguousarray(bk[cs].reshape(MT, P).T),
            "bv": np.tile(bv[cs], (P, 1)),
        })
    return in_maps


def kernel(x, Wq, bq, Wk, bk, Wv, bv, Wo, bo):
    x = np.asarray(x, dtype=np.float32)
    Wq, bq = np.asarray(Wq, np.float32), np.asarray(bq, np.float32)
    Wk, bk = np.asarray(Wk, np.float32), np.asarray(bk, np.float32)
    Wv, bv = np.asarray(Wv, np.float32), np.asarray(bv, np.float32)
    Wo, bo = np.asarray(Wo, np.float32), np.asarray(bo, np.float32)

    if "nc" not in _CACHE:
        _CACHE["nc"] = build_nc()
    nc = _CACHE["nc"]

    in_maps = _prep_inputs(x, Wq, bq, Wk, bk, Wv, bv, Wo, bo)
    res = run_bass_kernel_spmd(nc, in_maps, core_ids=list(range(8))).results

    out = np.empty((B, S, D), dtype=np.float32)
    for b in range(B):
        acc = res[4 * b]["out"].copy()
        for g in range(1, 4):
            acc += res[4 * b + g]["out"]
        out[b] = acc + bo
    return out

